# revision 30
# baseline (speedup 1.0000x reference)
"""Linear self-attention (elu+1 feature map) Trainium2 kernel.

Problem: B=4, S=4096, D=1024, H=16, HD=64.
  q = elu1(x @ Wq.T + bq); k = elu1(x @ Wk.T + bk); v = x @ Wv.T + bv
  kv_h = k_h^T v_h; ksum_h = sum_t k_h; z = 1/(q.ksum + eps)
  out = (q_h @ kv_h) * z; y = out @ Wo.T + bo

Sharding: token-parallel. Core c handles batch c//2, sequence half c%2
(2048 tokens). kv/ksum are partial sums over local tokens, AllReduced
across the 2-core group sharing a batch, then every core finishes its
own tokens through attention + output projection. bo is added on host.

x is shipped pre-transposed (chunk-major x^T) so the projection
matmuls need no on-device transpose of x.
"""

import numpy as np
from contextlib import ExitStack

import concourse.bass as bass
import concourse.tile as tile
from concourse import bacc, mybir
from concourse.bass_utils import run_bass_kernel_spmd

B, S, D, H, HD = 4, 4096, 1024, 16, 64
N_CORES = 8
TOK = (B * S) // N_CORES      # 2048 tokens per core
NT = TOK // 128               # 16 token tiles per core
F32 = mybir.dt.float32
F32R = mybir.dt.float32r
BF16 = mybir.dt.bfloat16
EPS = 1e-6

# dtype for the big matmuls (projections, attention, output projection).
# float32r = relaxed-precision fp32 (4-byte storage), bfloat16 = 2-byte.
MM_DT = F32R

TRACE = False            # set by test harness for profiling
LAST_RESULT = None       # BassKernelResults of last run
DEBUG = False            # dump tile-0 intermediates to extra outputs

_PROGRAMS = {}


def _emit(nc, has_bias, mm_dt):
    AF = mybir.ActivationFunctionType
    ALU = mybir.AluOpType
    esz = mybir.dt.size(mm_dt)

    # x^T, chunk-major per token tile: [p, t*1024 + c*128 + j] =
    #   x[t*128 + j, c*128 + p]
    xst = nc.dram_tensor("xst", [128, NT * 1024], mm_dt, kind="ExternalInput").ap()
    wkd = nc.dram_tensor("wkt", [128, 8 * D], mm_dt, kind="ExternalInput").ap()
    wvd = nc.dram_tensor("wvt", [128, 8 * D], mm_dt, kind="ExternalInput").ap()
    wqd = nc.dram_tensor("wqt", [128, 8 * D], mm_dt, kind="ExternalInput").ap()
    wod = nc.dram_tensor("wot", [128, 8 * D], mm_dt, kind="ExternalInput").ap()
    identd = nc.dram_tensor("ident", [128, 128], F32, kind="ExternalInput").ap()
    biasd = nc.dram_tensor("biases", [1, 4096], F32, kind="ExternalInput").ap()
    y_d = nc.dram_tensor("y", [TOK, D], F32, kind="ExternalOutput").ap()
    qspill = nc.dram_tensor("qspill", [128, NT * 1024], mm_dt).ap()
    cc_in = nc.dram_tensor("cc_in", [64, 1040], F32).ap()
    cc_out = nc.dram_tensor("cc_out", [64, 1040], F32).ap()

    dbg = {}
    if DEBUG:
        for name, shape in [("dbg_k", [128, 1024]), ("dbg_v", [128, 1024]),
                            ("dbg_q", [128, 1024]),
                            ("dbg_kv", [64, 1040]), ("dbg_bd", [128, 2048]),
                            ("dbg_od", [128, 2048]), ("dbg_osb", [128, 1024])]:
            dbg[name] = nc.dram_tensor(name, shape, F32, kind="ExternalOutput").ap()

    with tile.TileContext(nc) as tc, ExitStack() as top:
        wpool = top.enter_context(tc.tile_pool(name="w", bufs=3))
        cpool = top.enter_context(tc.tile_pool(name="const", bufs=1))
        ident = cpool.tile([128, 128], F32, tag="ident")
        nc.sync.dma_start(ident[:], identd)
        ones = cpool.tile([128, 1], F32, tag="ones")
        nc.vector.memset(ones[:], 1.0)
        if has_bias:
            ones_row_st = cpool.tile([1, 128], F32, tag="ones_row_st")
            nc.vector.memset(ones_row_st[:], 1.0)
            ones_row = cpool.tile([1, 128], mm_dt, tag="ones_row")
            nc.vector.tensor_copy(ones_row[:], ones_row_st[:])
            bias_st = cpool.tile([1, 3072], F32, tag="bias_st")
            nc.sync.dma_start(bias_st[:], biasd[0:1, 0:3072])
            bias_sb = cpool.tile([1, 3072], mm_dt, tag="bias")
            nc.vector.tensor_copy(bias_sb[:], bias_st[:])

        def load_weight(dram_ap):
            # weights go via SWDGE so the x-tile DMAs on the sync queue
            # aren't stuck behind 16 MiB of weights at kernel start
            wt = wpool.tile([128, 8 * D], mm_dt, tag="w")
            nc.gpsimd.dma_start(wt[:], dram_ap)
            return wt

        wk_t = load_weight(wkd)
        wv_t = load_weight(wvd)
        wq_t = load_weight(wqd)

        kvstack = ExitStack()
        kvpool = kvstack.enter_context(tc.tile_pool(name="kvp", bufs=1, space="PSUM"))
        kv_ps = kvpool.tile([64, 1024], F32, tag="kv")
        ksum_ps = kvpool.tile([64, 16], F32, tag="ksum")

        # ---------------- Pass 1: q/k/v projections, kv + ksum ----------------
        with ExitStack() as p1:
            xtpool = p1.enter_context(tc.tile_pool(name="xt", bufs=3))
            kqv_pool = p1.enter_context(tc.tile_pool(name="kqv", bufs=2))
            mepool = p1.enter_context(tc.tile_pool(name="me", bufs=2))
            qtpool = p1.enter_context(tc.tile_pool(name="qt", bufs=2))
            projp = p1.enter_context(tc.tile_pool(name="projp", bufs=2, space="PSUM"))
            tpp = p1.enter_context(tc.tile_pool(name="tpp", bufs=1, space="PSUM"))

            def add_bias(ps, boff, g):
                if has_bias:
                    nc.tensor.matmul(
                        ps[:, g * 512:(g + 1) * 512],
                        ones_row[0:1, 0:128],
                        bias_sb[0:1, boff + g * 512: boff + g * 512 + 512],
                        start=False, stop=True,
                    )

            def elu1(dst, ps):
                # elu(x)+1 = exp(min(x,0)) + max(x,0)
                me = mepool.tile([128, 1024], F32, tag="me")
                nc.vector.tensor_scalar_min(me[:], ps[:], 0.0)
                nc.scalar.activation(me[:], me[:], AF.Exp)
                nc.vector.scalar_tensor_tensor(
                    dst[:], ps[:], 0.0, me[:], ALU.max, ALU.add)

            for t in range(NT):
                xt = xtpool.tile([128, 1024], mm_dt, tag="xt")
                nc.sync.dma_start(xt[:], xst[:, t * 1024:(t + 1) * 1024])

                # k+v pass, c-outer so consecutive matmuls share the
                # stationary operand (xt chunk)
                kps = projp.tile([128, 1024], F32, tag="proj")
                vps = projp.tile([128, 1024], F32, tag="proj")
                for c in range(8):
                    lhs = xt[:, c * 128:(c + 1) * 128]
                    for g in range(2):
                        nc.tensor.matmul(
                            kps[:, g * 512:(g + 1) * 512], lhs,
                            wk_t[:, c * D + g * 512: c * D + g * 512 + 512],
                            start=(c == 0), stop=(c == 7 and not has_bias))
                        nc.tensor.matmul(
                            vps[:, g * 512:(g + 1) * 512], lhs,
                            wv_t[:, c * D + g * 512: c * D + g * 512 + 512],
                            start=(c == 0), stop=(c == 7 and not has_bias))
                add_bias(kps, 1024, 0); add_bias(kps, 1024, 1)
                add_bias(vps, 2048, 0); add_bias(vps, 2048, 1)

                ksb = kqv_pool.tile([128, 1024], F32, tag="k")
                vsb = kqv_pool.tile([128, 1024], F32, tag="v")
                qsb = kqv_pool.tile([128, 1024], F32, tag="q")
                elu1(ksb, kps)
                nc.vector.tensor_copy(vsb[:], vps[:])

                # q pass (reuses a proj psum slot once kps is consumed)
                qps = projp.tile([128, 1024], F32, tag="proj")
                for c in range(8):
                    lhs = xt[:, c * 128:(c + 1) * 128]
                    for g in range(2):
                        nc.tensor.matmul(
                            qps[:, g * 512:(g + 1) * 512], lhs,
                            wq_t[:, c * D + g * 512: c * D + g * 512 + 512],
                            start=(c == 0), stop=(c == 7 and not has_bias))
                add_bias(qps, 0, 0); add_bias(qps, 0, 1)
                elu1(qsb, qps)

                for h in range(16):
                    # NOTE: start=True clears has_written for the whole PSUM
                    # bank, so only the first matmul per bank may set it.
                    nc.tensor.matmul(
                        kv_ps[0:64, h * 64:(h + 1) * 64],
                        ksb[:, h * 64:(h + 1) * 64],
                        vsb[:, h * 64:(h + 1) * 64],
                        start=(t == 0 and h % 8 == 0), stop=(t == NT - 1),
                    )
                    nc.tensor.matmul(
                        ksum_ps[0:64, h:h + 1],
                        ksb[:, h * 64:(h + 1) * 64],
                        ones[:, 0:1],
                        start=(t == 0 and h == 0), stop=(t == NT - 1),
                    )

                qt = qtpool.tile([128, 1024], mm_dt, tag="qt")
                for c in range(8):
                    tp_ps = tpp.tile([128, 128], F32, tag="tp")
                    nc.tensor.transpose(tp_ps[:], qsb[:, c * 128:(c + 1) * 128], ident[:])
                    nc.vector.tensor_copy(qt[:, c * 128:(c + 1) * 128], tp_ps[:])
                nc.sync.dma_start(qspill[:, t * 1024:(t + 1) * 1024], qt[:])

                if DEBUG and t == 0:
                    nc.sync.dma_start(dbg["dbg_k"][:], ksb[:])
                    nc.sync.dma_start(dbg["dbg_v"][:], vsb[:])
                    nc.sync.dma_start(dbg["dbg_q"][:], qsb[:])

            kvsb = cpool.tile([64, 1040], F32, tag="kvsb")
            nc.vector.tensor_copy(kvsb[:, 0:1024], kv_ps[:])
            nc.vector.tensor_copy(kvsb[:, 1024:1040], ksum_ps[:])
            nc.sync.dma_start(cc_in[:], kvsb[:])
            if DEBUG:
                nc.sync.dma_start(dbg["dbg_kv"][:], kvsb[:])

        kvstack.close()

        nc.gpsimd.collective_compute(
            "AllReduce", mybir.AluOpType.add,
            replica_groups=[[0, 1], [2, 3], [4, 5], [6, 7]],
            ins=[cc_in[:]], outs=[cc_out[:]],
        )

        # ---------------- Pass 2: attention + output projection ----------------
        with ExitStack() as p2:
            bdpool = p2.enter_context(tc.tile_pool(name="bd", bufs=1))
            qtip = p2.enter_context(tc.tile_pool(name="qti", bufs=3))
            out_pool = p2.enter_context(tc.tile_pool(name="osb", bufs=2))
            outT_pool = p2.enter_context(tc.tile_pool(name="otb", bufs=2))
            y_pool = p2.enter_context(tc.tile_pool(name="ysb", bufs=2))
            zpool = p2.enter_context(tc.tile_pool(name="z", bufs=2))
            odp = p2.enter_context(tc.tile_pool(name="odp", bufs=2, space="PSUM"))
            tpp2 = p2.enter_context(tc.tile_pool(name="tpp2", bufs=2, space="PSUM"))
            ypp = p2.enter_context(tc.tile_pool(name="ypp", bufs=1, space="PSUM"))

            wo_t = load_weight(wod)

            # block-diagonal [kv | ksum] matrix: chunk c (heads 2c, 2c+1):
            # rows 0:64 = head 2c (d), rows 64:128 = head 2c+1
            # cols c*256+[0:64] = kv_2c, [64:128] = kv_2c+1, 128/129 = ksums
            bd_st = bdpool.tile([128, 2048], F32, tag="bd_st")
            nc.vector.memset(bd_st[:], 0.0)
            cc_kv = cc_out[0:64, 0:1024].rearrange("p (j i e) -> p i j e", i=2, e=64)
            bd_lo = bd_st[0:64, :].rearrange("p (c r) -> p c r", r=256)
            bd_hi = bd_st[64:128, :].rearrange("p (c r) -> p c r", r=256)
            nc.sync.dma_start(bd_lo[:, :, 0:64], cc_kv[:, 0, :, :])
            nc.sync.dma_start(bd_hi[:, :, 64:128], cc_kv[:, 1, :, :])
            cc_ks = cc_out[0:64, 1024:1040].rearrange("p (j i) -> p j i", i=2)
            nc.sync.dma_start(bd_lo[:, :, 128:129], cc_ks[:, :, 0:1])
            nc.sync.dma_start(bd_hi[:, :, 129:130], cc_ks[:, :, 1:2])
            bd = bdpool.tile([128, 2048], mm_dt, tag="bd")
            nc.vector.tensor_copy(bd[:], bd_st[:])
            if DEBUG:
                nc.sync.dma_start(dbg["dbg_bd"][:], bd_st[:])

            # software pipeline: stage A(t) = qti DMA + attention matmuls,
            # stage B(t) = normalize/scale + transpose + output projection.
            # A(t+1) is emitted before B(t) so the PE has matmul work while
            # the vector engine normalizes tile t.
            state = {}

            def stage_a(t):
                qti = qtip.tile([128, 1024], mm_dt, tag="qti")
                nc.sync.dma_start(qti[:], qspill[:, t * 1024:(t + 1) * 1024])
                ods = [odp.tile([128, 1024], F32, tag="od", name=f"od{t}_{i}")
                       for i in range(2)]
                zden = zpool.tile([128, 16], F32, tag="zden")
                for half in range(2):
                    od = ods[half]
                    for cc in range(4):
                        c = half * 4 + cc
                        nc.tensor.matmul(
                            od[:, cc * 256:(cc + 1) * 256],
                            qti[:, c * 128:(c + 1) * 128],
                            bd[:, c * 256:(c + 1) * 256],
                            start=True, stop=True,
                        )
                    od_r = od[:].rearrange("p (c r) -> p c r", r=256)
                    nc.vector.tensor_copy(
                        zden[:, half * 8:(half + 1) * 8]
                        .rearrange("p (c i) -> p c i", i=2),
                        od_r[:, :, 128:130])
                state[t] = (ods, zden)

            def stage_b(t):
                ods, zden = state.pop(t)
                zinv = zpool.tile([128, 16], F32, tag="zinv")
                nc.vector.tensor_scalar_add(zden[:], zden[:], EPS)
                nc.vector.reciprocal(zinv[:], zden[:])
                osb = out_pool.tile([128, 1024], F32, tag="osb")
                for half in range(2):
                    od_r = ods[half][:].rearrange("p (c r) -> p c r", r=256)
                    zb = (zinv[:, half * 8:(half + 1) * 8]
                          .rearrange("p (c i) -> p c i", i=2)
                          .unsqueeze(3).broadcast_to((128, 4, 2, 64)))
                    nc.vector.tensor_mul(
                        osb[:, half * 512:(half + 1) * 512]
                        .rearrange("p (c i e) -> p c i e", c=4, i=2),
                        od_r[:, :, 0:128].rearrange("p c (i e) -> p c i e", i=2),
                        zb,
                    )

                otb = outT_pool.tile([128, 1024], mm_dt, tag="otb")
                for c in range(8):
                    tp2 = tpp2.tile([128, 128], F32, tag="tp2")
                    nc.tensor.transpose(tp2[:], osb[:, c * 128:(c + 1) * 128], ident[:])
                    nc.vector.tensor_copy(otb[:, c * 128:(c + 1) * 128], tp2[:])

                yps = ypp.tile([128, 1024], F32, tag="y")
                for c in range(8):
                    lhs = otb[:, c * 128:(c + 1) * 128]
                    for g in range(2):
                        nc.tensor.matmul(
                            yps[:, g * 512:(g + 1) * 512], lhs,
                            wo_t[:, c * D + g * 512: c * D + g * 512 + 512],
                            start=(c == 0), stop=(c == 7),
                        )
                ysb = y_pool.tile([128, 1024], F32, tag="ysb")
                nc.vector.tensor_copy(ysb[:], yps[:])
                nc.sync.dma_start(y_d[t * 128:(t + 1) * 128, :], ysb[:])

                if DEBUG and t == 0:
                    for half in range(2):
                        odsb = y_pool.tile([128, 1024], F32, tag="odsb",
                                           name=f"odsb{half}")
                        nc.vector.tensor_copy(odsb[:], ods[half][:])
                        nc.sync.dma_start(
                            dbg["dbg_od"][:, half * 1024:(half + 1) * 1024], odsb[:])
                    nc.sync.dma_start(dbg["dbg_osb"][:], osb[:])

            stage_a(0)
            for t in range(1, NT):
                stage_a(t)
                stage_b(t - 1)
            stage_b(NT - 1)


def _get_program(has_bias):
    key = (has_bias, MM_DT)
    if key not in _PROGRAMS:
        nc = bacc.Bacc("TRN2", target_bir_lowering=False, debug=False,
                       num_devices=N_CORES)
        _emit(nc, has_bias, MM_DT)
        nc.compile()
        _PROGRAMS[key] = nc
    return _PROGRAMS[key]


def _to_mm_np(a):
    """Convert fp32 array to the numpy dtype matching MM_DT."""
    if MM_DT == BF16:
        import ml_dtypes
        return np.ascontiguousarray(a.astype(ml_dtypes.bfloat16))
    return np.ascontiguousarray(a)


def _pack_rhs(w):
    # W [out,in] -> W.T chunk-major rhs layout [128, 8*1024]:
    # [p, c*1024 + n] = W.T[c*128+p, n]
    return _to_mm_np(
        w.T.reshape(8, 128, D).transpose(1, 0, 2).reshape(128, 8 * D))


def _pack_xt(xs):
    # xs [TOK, D] -> x^T tile-major: [p, t*1024 + c*128 + j] = xs[t*128+j, c*128+p]
    return _to_mm_np(
        xs.T.reshape(8, 128, NT, 128).transpose(1, 2, 0, 3).reshape(128, NT * 1024))


def kernel(x, Wq, bq, Wk, bk, Wv, bv, Wo, bo):
    global LAST_RESULT
    x = np.asarray(x, dtype=np.float32)
    Wq, Wk, Wv, Wo = (np.asarray(w, dtype=np.float32) for w in (Wq, Wk, Wv, Wo))
    bq, bk, bv, bo = (np.asarray(b, dtype=np.float32) for b in (bq, bk, bv, bo))

    has_bias = bool(np.any(bq) or np.any(bk) or np.any(bv))
    nc = _get_program(has_bias)
    shared = {
        "wkt": _pack_rhs(Wk),
        "wvt": _pack_rhs(Wv),
        "wqt": _pack_rhs(Wq),
        "wot": _pack_rhs(Wo),
        "ident": np.eye(128, dtype=np.float32),
        "biases": np.concatenate([bq, bk, bv, bo]).reshape(1, 4096),
    }
    in_maps = []
    for c in range(N_CORES):
        b = c // 2
        h = c % 2
        m = dict(shared)
        m["xst"] = _pack_xt(x[b, h * TOK:(h + 1) * TOK, :])
        in_maps.append(m)

    res = run_bass_kernel_spmd(nc, in_maps, list(range(N_CORES)), trace=TRACE)
    LAST_RESULT = res

    y = np.empty((B, S, D), dtype=np.float32)
    for c in range(N_CORES):
        b = c // 2
        h = c % 2
        y[b, h * TOK:(h + 1) * TOK, :] = res.results[c]["y"]
    y += bo
    return y


# revision 31
# speedup vs baseline: 1.0953x; 1.0953x over previous
"""Linear self-attention (elu+1 feature map) Trainium2 kernel.

Problem: B=4, S=4096, D=1024, H=16, HD=64.
  q = elu1(x @ Wq.T + bq); k = elu1(x @ Wk.T + bk); v = x @ Wv.T + bv
  kv_h = k_h^T v_h; ksum_h = sum_t k_h; z = 1/(q.ksum + eps)
  out = (q_h @ kv_h) * z; y = out @ Wo.T + bo

Sharding: token-parallel. Core c handles batch c//2, sequence half c%2
(2048 tokens). kv/ksum are partial sums over local tokens, AllReduced
across the 2-core group sharing a batch, then every core finishes its
own tokens through attention + output projection. bo is added on host.

x is shipped pre-transposed (chunk-major x^T) so the projection
matmuls need no on-device transpose of x.
"""

import numpy as np
from contextlib import ExitStack

import concourse.bass as bass
import concourse.tile as tile
from concourse import bacc, mybir
from concourse.bass_utils import run_bass_kernel_spmd

B, S, D, H, HD = 4, 4096, 1024, 16, 64
N_CORES = 8
TOK = (B * S) // N_CORES      # 2048 tokens per core
NT = TOK // 128               # 16 token tiles per core
F32 = mybir.dt.float32
F32R = mybir.dt.float32r
BF16 = mybir.dt.bfloat16
EPS = 1e-6

# dtype for the big matmuls (projections, attention, output projection).
# float32r = relaxed-precision fp32 (4-byte storage), bfloat16 = 2-byte.
MM_DT = BF16

TRACE = False            # set by test harness for profiling
LAST_RESULT = None       # BassKernelResults of last run
DEBUG = False            # dump tile-0 intermediates to extra outputs

_PROGRAMS = {}


def _emit(nc, has_bias, mm_dt):
    AF = mybir.ActivationFunctionType
    ALU = mybir.AluOpType
    esz = mybir.dt.size(mm_dt)

    # x^T, chunk-major per token tile: [p, t*1024 + c*128 + j] =
    #   x[t*128 + j, c*128 + p]
    xst = nc.dram_tensor("xst", [128, NT * 1024], mm_dt, kind="ExternalInput").ap()
    wkd = nc.dram_tensor("wkt", [128, 8 * D], mm_dt, kind="ExternalInput").ap()
    wvd = nc.dram_tensor("wvt", [128, 8 * D], mm_dt, kind="ExternalInput").ap()
    wqd = nc.dram_tensor("wqt", [128, 8 * D], mm_dt, kind="ExternalInput").ap()
    wod = nc.dram_tensor("wot", [128, 8 * D], mm_dt, kind="ExternalInput").ap()
    identd = nc.dram_tensor("ident", [128, 128], F32, kind="ExternalInput").ap()
    biasd = nc.dram_tensor("biases", [1, 4096], F32, kind="ExternalInput").ap()
    y_d = nc.dram_tensor("y", [TOK, D], F32, kind="ExternalOutput").ap()
    qspill = nc.dram_tensor("qspill", [128, NT * 1024], mm_dt).ap()
    cc_in = nc.dram_tensor("cc_in", [64, 1040], F32).ap()
    cc_out = nc.dram_tensor("cc_out", [64, 1040], F32).ap()

    dbg = {}
    if DEBUG:
        for name, shape in [("dbg_k", [128, 1024]), ("dbg_v", [128, 1024]),
                            ("dbg_q", [128, 1024]),
                            ("dbg_kv", [64, 1040]), ("dbg_bd", [128, 2048]),
                            ("dbg_od", [128, 2048]), ("dbg_osb", [128, 1024])]:
            dbg[name] = nc.dram_tensor(name, shape, F32, kind="ExternalOutput").ap()

    with tile.TileContext(nc) as tc, ExitStack() as top:
        wpool = top.enter_context(tc.tile_pool(name="w", bufs=3))
        cpool = top.enter_context(tc.tile_pool(name="const", bufs=1))
        ident = cpool.tile([128, 128], F32, tag="ident")
        nc.sync.dma_start(ident[:], identd)
        ones = cpool.tile([128, 1], F32, tag="ones")
        nc.vector.memset(ones[:], 1.0)
        if has_bias:
            ones_row_st = cpool.tile([1, 128], F32, tag="ones_row_st")
            nc.vector.memset(ones_row_st[:], 1.0)
            ones_row = cpool.tile([1, 128], mm_dt, tag="ones_row")
            nc.vector.tensor_copy(ones_row[:], ones_row_st[:])
            bias_st = cpool.tile([1, 3072], F32, tag="bias_st")
            nc.sync.dma_start(bias_st[:], biasd[0:1, 0:3072])
            bias_sb = cpool.tile([1, 3072], mm_dt, tag="bias")
            nc.vector.tensor_copy(bias_sb[:], bias_st[:])

        def load_weight(dram_ap):
            # weights go via SWDGE so the x-tile DMAs on the sync queue
            # aren't stuck behind 16 MiB of weights at kernel start
            wt = wpool.tile([128, 8 * D], mm_dt, tag="w")
            nc.gpsimd.dma_start(wt[:], dram_ap)
            return wt

        wk_t = load_weight(wkd)
        wv_t = load_weight(wvd)
        wq_t = load_weight(wqd)

        kvstack = ExitStack()
        kvpool = kvstack.enter_context(tc.tile_pool(name="kvp", bufs=1, space="PSUM"))
        kv_ps = kvpool.tile([64, 1024], F32, tag="kv")
        ksum_ps = kvpool.tile([64, 16], F32, tag="ksum")

        # ---------------- Pass 1: q/k/v projections, kv + ksum ----------------
        with ExitStack() as p1:
            xtpool = p1.enter_context(tc.tile_pool(name="xt", bufs=3))
            kqv_pool = p1.enter_context(tc.tile_pool(name="kqv", bufs=2))
            mepool = p1.enter_context(tc.tile_pool(name="me", bufs=2))
            qtpool = p1.enter_context(tc.tile_pool(name="qt", bufs=2))
            projp = p1.enter_context(tc.tile_pool(name="projp", bufs=2, space="PSUM"))
            tpp = p1.enter_context(tc.tile_pool(name="tpp", bufs=1, space="PSUM"))

            def add_bias(ps, boff, g):
                if has_bias:
                    nc.tensor.matmul(
                        ps[:, g * 512:(g + 1) * 512],
                        ones_row[0:1, 0:128],
                        bias_sb[0:1, boff + g * 512: boff + g * 512 + 512],
                        start=False, stop=True,
                    )

            def elu1(dst, ps):
                # elu(x)+1 = exp(min(x,0)) + max(x,0)
                me = mepool.tile([128, 1024], F32, tag="me")
                nc.vector.tensor_scalar_min(me[:], ps[:], 0.0)
                nc.scalar.activation(me[:], me[:], AF.Exp)
                nc.vector.scalar_tensor_tensor(
                    dst[:], ps[:], 0.0, me[:], ALU.max, ALU.add)

            for t in range(NT):
                xt = xtpool.tile([128, 1024], mm_dt, tag="xt")
                nc.sync.dma_start(xt[:], xst[:, t * 1024:(t + 1) * 1024])

                # k+v pass, c-outer so consecutive matmuls share the
                # stationary operand (xt chunk)
                kps = projp.tile([128, 1024], F32, tag="proj")
                vps = projp.tile([128, 1024], F32, tag="proj")
                for c in range(8):
                    lhs = xt[:, c * 128:(c + 1) * 128]
                    for g in range(2):
                        nc.tensor.matmul(
                            kps[:, g * 512:(g + 1) * 512], lhs,
                            wk_t[:, c * D + g * 512: c * D + g * 512 + 512],
                            start=(c == 0), stop=(c == 7 and not has_bias))
                        nc.tensor.matmul(
                            vps[:, g * 512:(g + 1) * 512], lhs,
                            wv_t[:, c * D + g * 512: c * D + g * 512 + 512],
                            start=(c == 0), stop=(c == 7 and not has_bias))
                add_bias(kps, 1024, 0); add_bias(kps, 1024, 1)
                add_bias(vps, 2048, 0); add_bias(vps, 2048, 1)

                ksb = kqv_pool.tile([128, 1024], F32, tag="k")
                vsb = kqv_pool.tile([128, 1024], F32, tag="v")
                qsb = kqv_pool.tile([128, 1024], F32, tag="q")
                elu1(ksb, kps)
                nc.vector.tensor_copy(vsb[:], vps[:])

                # q pass (reuses a proj psum slot once kps is consumed)
                qps = projp.tile([128, 1024], F32, tag="proj")
                for c in range(8):
                    lhs = xt[:, c * 128:(c + 1) * 128]
                    for g in range(2):
                        nc.tensor.matmul(
                            qps[:, g * 512:(g + 1) * 512], lhs,
                            wq_t[:, c * D + g * 512: c * D + g * 512 + 512],
                            start=(c == 0), stop=(c == 7 and not has_bias))
                add_bias(qps, 0, 0); add_bias(qps, 0, 1)
                elu1(qsb, qps)

                for h in range(16):
                    # NOTE: start=True clears has_written for the whole PSUM
                    # bank, so only the first matmul per bank may set it.
                    nc.tensor.matmul(
                        kv_ps[0:64, h * 64:(h + 1) * 64],
                        ksb[:, h * 64:(h + 1) * 64],
                        vsb[:, h * 64:(h + 1) * 64],
                        start=(t == 0 and h % 8 == 0), stop=(t == NT - 1),
                    )
                    nc.tensor.matmul(
                        ksum_ps[0:64, h:h + 1],
                        ksb[:, h * 64:(h + 1) * 64],
                        ones[:, 0:1],
                        start=(t == 0 and h == 0), stop=(t == NT - 1),
                    )

                qt = qtpool.tile([128, 1024], mm_dt, tag="qt")
                for c in range(8):
                    tp_ps = tpp.tile([128, 128], F32, tag="tp")
                    nc.tensor.transpose(tp_ps[:], qsb[:, c * 128:(c + 1) * 128], ident[:])
                    nc.vector.tensor_copy(qt[:, c * 128:(c + 1) * 128], tp_ps[:])
                nc.sync.dma_start(qspill[:, t * 1024:(t + 1) * 1024], qt[:])

                if DEBUG and t == 0:
                    nc.sync.dma_start(dbg["dbg_k"][:], ksb[:])
                    nc.sync.dma_start(dbg["dbg_v"][:], vsb[:])
                    nc.sync.dma_start(dbg["dbg_q"][:], qsb[:])

            kvsb = cpool.tile([64, 1040], F32, tag="kvsb")
            nc.vector.tensor_copy(kvsb[:, 0:1024], kv_ps[:])
            nc.vector.tensor_copy(kvsb[:, 1024:1040], ksum_ps[:])
            nc.sync.dma_start(cc_in[:], kvsb[:])
            if DEBUG:
                nc.sync.dma_start(dbg["dbg_kv"][:], kvsb[:])

        kvstack.close()

        nc.gpsimd.collective_compute(
            "AllReduce", mybir.AluOpType.add,
            replica_groups=[[0, 1], [2, 3], [4, 5], [6, 7]],
            ins=[cc_in[:]], outs=[cc_out[:]],
        )

        # ---------------- Pass 2: attention + output projection ----------------
        with ExitStack() as p2:
            bdpool = p2.enter_context(tc.tile_pool(name="bd", bufs=1))
            qtip = p2.enter_context(tc.tile_pool(name="qti", bufs=3))
            out_pool = p2.enter_context(tc.tile_pool(name="osb", bufs=2))
            outT_pool = p2.enter_context(tc.tile_pool(name="otb", bufs=2))
            y_pool = p2.enter_context(tc.tile_pool(name="ysb", bufs=2))
            zpool = p2.enter_context(tc.tile_pool(name="z", bufs=2))
            odp = p2.enter_context(tc.tile_pool(name="odp", bufs=2, space="PSUM"))
            tpp2 = p2.enter_context(tc.tile_pool(name="tpp2", bufs=2, space="PSUM"))
            ypp = p2.enter_context(tc.tile_pool(name="ypp", bufs=1, space="PSUM"))

            wo_t = load_weight(wod)

            # block-diagonal [kv | ksum] matrix: chunk c (heads 2c, 2c+1):
            # rows 0:64 = head 2c (d), rows 64:128 = head 2c+1
            # cols c*256+[0:64] = kv_2c, [64:128] = kv_2c+1, 128/129 = ksums
            bd_st = bdpool.tile([128, 2048], F32, tag="bd_st")
            nc.vector.memset(bd_st[:], 0.0)
            cc_kv = cc_out[0:64, 0:1024].rearrange("p (j i e) -> p i j e", i=2, e=64)
            bd_lo = bd_st[0:64, :].rearrange("p (c r) -> p c r", r=256)
            bd_hi = bd_st[64:128, :].rearrange("p (c r) -> p c r", r=256)
            nc.sync.dma_start(bd_lo[:, :, 0:64], cc_kv[:, 0, :, :])
            nc.sync.dma_start(bd_hi[:, :, 64:128], cc_kv[:, 1, :, :])
            cc_ks = cc_out[0:64, 1024:1040].rearrange("p (j i) -> p j i", i=2)
            nc.sync.dma_start(bd_lo[:, :, 128:129], cc_ks[:, :, 0:1])
            nc.sync.dma_start(bd_hi[:, :, 129:130], cc_ks[:, :, 1:2])
            bd = bdpool.tile([128, 2048], mm_dt, tag="bd")
            nc.vector.tensor_copy(bd[:], bd_st[:])
            if DEBUG:
                nc.sync.dma_start(dbg["dbg_bd"][:], bd_st[:])

            # software pipeline: stage A(t) = qti DMA + attention matmuls,
            # stage B(t) = normalize/scale + transpose + output projection.
            # A(t+1) is emitted before B(t) so the PE has matmul work while
            # the vector engine normalizes tile t.
            state = {}

            def stage_a(t):
                qti = qtip.tile([128, 1024], mm_dt, tag="qti")
                nc.sync.dma_start(qti[:], qspill[:, t * 1024:(t + 1) * 1024])
                ods = [odp.tile([128, 1024], F32, tag="od", name=f"od{t}_{i}")
                       for i in range(2)]
                zden = zpool.tile([128, 16], F32, tag="zden")
                for half in range(2):
                    od = ods[half]
                    for cc in range(4):
                        c = half * 4 + cc
                        nc.tensor.matmul(
                            od[:, cc * 256:(cc + 1) * 256],
                            qti[:, c * 128:(c + 1) * 128],
                            bd[:, c * 256:(c + 1) * 256],
                            start=True, stop=True,
                        )
                    od_r = od[:].rearrange("p (c r) -> p c r", r=256)
                    nc.vector.tensor_copy(
                        zden[:, half * 8:(half + 1) * 8]
                        .rearrange("p (c i) -> p c i", i=2),
                        od_r[:, :, 128:130])
                state[t] = (ods, zden)

            def stage_b(t):
                ods, zden = state.pop(t)
                zinv = zpool.tile([128, 16], F32, tag="zinv")
                nc.vector.tensor_scalar_add(zden[:], zden[:], EPS)
                nc.vector.reciprocal(zinv[:], zden[:])
                osb = out_pool.tile([128, 1024], F32, tag="osb")
                for half in range(2):
                    od_r = ods[half][:].rearrange("p (c r) -> p c r", r=256)
                    zb = (zinv[:, half * 8:(half + 1) * 8]
                          .rearrange("p (c i) -> p c i", i=2)
                          .unsqueeze(3).broadcast_to((128, 4, 2, 64)))
                    nc.vector.tensor_mul(
                        osb[:, half * 512:(half + 1) * 512]
                        .rearrange("p (c i e) -> p c i e", c=4, i=2),
                        od_r[:, :, 0:128].rearrange("p c (i e) -> p c i e", i=2),
                        zb,
                    )

                otb = outT_pool.tile([128, 1024], mm_dt, tag="otb")
                for c in range(8):
                    tp2 = tpp2.tile([128, 128], F32, tag="tp2")
                    nc.tensor.transpose(tp2[:], osb[:, c * 128:(c + 1) * 128], ident[:])
                    nc.vector.tensor_copy(otb[:, c * 128:(c + 1) * 128], tp2[:])

                yps = ypp.tile([128, 1024], F32, tag="y")
                for c in range(8):
                    lhs = otb[:, c * 128:(c + 1) * 128]
                    for g in range(2):
                        nc.tensor.matmul(
                            yps[:, g * 512:(g + 1) * 512], lhs,
                            wo_t[:, c * D + g * 512: c * D + g * 512 + 512],
                            start=(c == 0), stop=(c == 7),
                        )
                ysb = y_pool.tile([128, 1024], F32, tag="ysb")
                nc.vector.tensor_copy(ysb[:], yps[:])
                nc.sync.dma_start(y_d[t * 128:(t + 1) * 128, :], ysb[:])

                if DEBUG and t == 0:
                    for half in range(2):
                        odsb = y_pool.tile([128, 1024], F32, tag="odsb",
                                           name=f"odsb{half}")
                        nc.vector.tensor_copy(odsb[:], ods[half][:])
                        nc.sync.dma_start(
                            dbg["dbg_od"][:, half * 1024:(half + 1) * 1024], odsb[:])
                    nc.sync.dma_start(dbg["dbg_osb"][:], osb[:])

            stage_a(0)
            for t in range(1, NT):
                stage_a(t)
                stage_b(t - 1)
            stage_b(NT - 1)


def _get_program(has_bias):
    key = (has_bias, MM_DT)
    if key not in _PROGRAMS:
        nc = bacc.Bacc("TRN2", target_bir_lowering=False, debug=False,
                       num_devices=N_CORES)
        _emit(nc, has_bias, MM_DT)
        nc.compile()
        _PROGRAMS[key] = nc
    return _PROGRAMS[key]


def _to_mm_np(a):
    """Convert fp32 array to the numpy dtype matching MM_DT."""
    if MM_DT == BF16:
        import ml_dtypes
        return np.ascontiguousarray(a.astype(ml_dtypes.bfloat16))
    return np.ascontiguousarray(a)


def _pack_rhs(w):
    # W [out,in] -> W.T chunk-major rhs layout [128, 8*1024]:
    # [p, c*1024 + n] = W.T[c*128+p, n]
    return _to_mm_np(
        w.T.reshape(8, 128, D).transpose(1, 0, 2).reshape(128, 8 * D))


def _pack_xt(xs):
    # xs [TOK, D] -> x^T tile-major: [p, t*1024 + c*128 + j] = xs[t*128+j, c*128+p]
    return _to_mm_np(
        xs.T.reshape(8, 128, NT, 128).transpose(1, 2, 0, 3).reshape(128, NT * 1024))


def kernel(x, Wq, bq, Wk, bk, Wv, bv, Wo, bo):
    global LAST_RESULT
    x = np.asarray(x, dtype=np.float32)
    Wq, Wk, Wv, Wo = (np.asarray(w, dtype=np.float32) for w in (Wq, Wk, Wv, Wo))
    bq, bk, bv, bo = (np.asarray(b, dtype=np.float32) for b in (bq, bk, bv, bo))

    has_bias = bool(np.any(bq) or np.any(bk) or np.any(bv))
    nc = _get_program(has_bias)
    shared = {
        "wkt": _pack_rhs(Wk),
        "wvt": _pack_rhs(Wv),
        "wqt": _pack_rhs(Wq),
        "wot": _pack_rhs(Wo),
        "ident": np.eye(128, dtype=np.float32),
        "biases": np.concatenate([bq, bk, bv, bo]).reshape(1, 4096),
    }
    in_maps = []
    for c in range(N_CORES):
        b = c // 2
        h = c % 2
        m = dict(shared)
        m["xst"] = _pack_xt(x[b, h * TOK:(h + 1) * TOK, :])
        in_maps.append(m)

    res = run_bass_kernel_spmd(nc, in_maps, list(range(N_CORES)), trace=TRACE)
    LAST_RESULT = res

    y = np.empty((B, S, D), dtype=np.float32)
    for c in range(N_CORES):
        b = c // 2
        h = c % 2
        y[b, h * TOK:(h + 1) * TOK, :] = res.results[c]["y"]
    y += bo
    return y
